# revision 3
# baseline (speedup 1.0000x reference)
"""DiagPooling (segment-reduce over square-image diagonals) on 8 NeuronCores.

Input  x: [8, 128, 512, 512] f32. Output: [8, 1, 513] f32 — per batch, the
mean over (channels, diagonal) of each diagonal offset in [-256, 256].

Sharding: batch b -> core b (data parallel, no communication).

Per-core algorithm (stride-513 trick): for a flat 512x512 image, the element
(i, j) sits at a = 512*i + j = 513*i + (j - i). So viewing the flat image as
rows of 513 (an overlapping, strided view), every diagonal becomes a COLUMN:
P[q, r] = flat[513*q + r] holds diagonal o = r (when q + r <= 511) or
o = r - 513 (when q + r >= 512). The needed diagonals o in [-256, 256] are
exactly {r <= 256, prefix rows} and {r >= 257, suffix rows}; a fixed 0/1 mask
(folded together with 1/(C*diag_len)) turns the segment reduce into masked
column sums. Channels are summed first (mask is channel-invariant), so the
mask is applied once to 1 MiB instead of 128 MiB.

SBUF layout: P rows q = g*128 + p -> partition p, free column g*513 + r
(4 row-groups side by side). The DMA reads each channel as 512 contiguous
2052-byte rows, so the load is at full HBM line rate.
"""

import numpy as np

import concourse.bass as bass
import concourse.bacc as bacc
import concourse.mybir as mybir
from concourse import tile
from concourse.bass_utils import run_bass_kernel_spmd

B, C, H = 8, 128, 512
R = H + 1               # 513: columns of the strided view
NG = 4                  # 512 q-rows split into 4 groups of 128 partitions
F = NG * R              # 2052: SBUF free width
CH_ELEMS = H * H        # elements per (b, c) image
PAD = H                 # tail pad so the last P-row read stays in bounds
N_IN = C * CH_ELEMS + PAD
F32 = mybir.dt.float32


def _build_weights() -> np.ndarray:
    """[128, F] f32: mask(q, r) / (C * diag_len(r)) in the SBUF tile layout."""
    q = np.arange(H, dtype=np.int64)[:, None]      # [512, 1]
    r = np.arange(R, dtype=np.int64)[None, :]      # [1, 513]
    prefix = (r <= H // 2) & (q + r <= H - 1)      # diagonal o = r
    suffix = (r > H // 2) & (q + r >= H) & (q <= H - 2)  # o = r - 513
    mask = prefix | suffix
    o = np.where(r <= H // 2, r, r - R)
    denom = float(C) * (H - np.abs(o)).astype(np.float64)
    w = mask.astype(np.float64) / denom            # [512, 513]
    return (
        w.reshape(NG, 128, R).transpose(1, 0, 2).reshape(128, F).astype(np.float32)
    )


def _build_program():
    nc = bacc.Bacc("TRN2", target_bir_lowering=False, debug=False, num_devices=B)
    xp = nc.dram_tensor("x", [N_IN], F32, kind="ExternalInput")
    wt = nc.dram_tensor("w", [128, F], F32, kind="ExternalInput")
    out_t = nc.dram_tensor("out", [1, R], F32, kind="ExternalOutput")

    with tile.TileContext(nc) as tc:
        with (
            tc.tile_pool(name="consts", bufs=1) as consts,
            tc.tile_pool(name="accp", bufs=1) as accp,
            tc.tile_pool(name="loadp", bufs=10) as loadp,
            tc.tile_pool(name="outp", bufs=1) as outp,
            tc.tile_pool(name="psum", bufs=2, space=bass.MemorySpace.PSUM) as psump,
        ):
            w_tile = consts.tile([128, F], F32)
            nc.sync.dma_start(out=w_tile[:], in_=wt.ap())
            ones = consts.tile([128, 1], F32)
            nc.vector.memset(ones[:], 1.0)

            acc = accp.tile([128, F], F32)
            for c in range(C):
                t = loadp.tile([128, F], F32)
                # strided view of channel c: (p, g, r) -> 513*(g*128 + p) + r
                src = bass.AP(
                    xp, c * CH_ELEMS, [[R, 128], [R * 128, NG], [1, R]]
                )
                dst = t[:].rearrange("p (g r) -> p g r", g=NG)
                nc.sync.dma_start(out=dst, in_=src)
                if c == 0:
                    nc.vector.tensor_copy(out=acc[:], in_=t[:])
                else:
                    nc.vector.tensor_add(out=acc[:], in0=acc[:], in1=t[:])

            # mask + 1/denominator, then fold the 4 q-groups together
            nc.vector.tensor_mul(out=acc[:], in0=acc[:], in1=w_tile[:])
            u = outp.tile([128, R], F32)
            nc.vector.tensor_add(out=u[:], in0=acc[:, 0:R], in1=acc[:, R : 2 * R])
            nc.vector.tensor_add(out=u[:], in0=u[:], in1=acc[:, 2 * R : 3 * R])
            nc.vector.tensor_add(out=u[:], in0=u[:], in1=acc[:, 3 * R : 4 * R])

            # partition reduction: ones[128,1]^T @ u[128, N] -> [1, N]
            ps_a = psump.tile([1, 512], F32)
            ps_b = psump.tile([1, 1], F32)
            nc.tensor.matmul(ps_a[:], ones[:], u[:, 0:512], start=True, stop=True)
            nc.tensor.matmul(ps_b[:], ones[:], u[:, 512:513], start=True, stop=True)
            res = outp.tile([1, R], F32)
            nc.vector.tensor_copy(out=res[:, 0:512], in_=ps_a[:])
            nc.vector.tensor_copy(out=res[:, 512:513], in_=ps_b[:])
            nc.sync.dma_start(out=out_t.ap(), in_=res[:])

    nc.compile()
    return nc


_CACHE = {}


def kernel(x, _trace=False, _trace_cores=None) -> np.ndarray:
    x = np.asarray(x, dtype=np.float32)
    assert x.shape == (B, C, H, H), x.shape

    if "nc" not in _CACHE:
        _CACHE["nc"] = _build_program()
        _CACHE["w"] = _build_weights()
    nc = _CACHE["nc"]
    w = _CACHE["w"]

    pad = np.zeros(PAD, dtype=np.float32)
    in_maps = [
        {"x": np.concatenate([np.ascontiguousarray(x[b]).reshape(-1), pad]), "w": w}
        for b in range(B)
    ]
    result = run_bass_kernel_spmd(
        nc,
        in_maps,
        core_ids=list(range(B)),
        trace=_trace,
        trace_cores=_trace_cores,
    )
    _CACHE["last_result"] = result

    out = np.empty((B, 1, R), dtype=np.float32)
    for b in range(B):
        r = result.results[b]["out"].reshape(R)
        # column r -> offset o = r (r <= 256) / r - 513 (r >= 257);
        # output index n = o + 256
        out[b, 0, :] = np.concatenate([r[R - 256 :], r[: R - 256]])
    return out
